# revision 10
# baseline (speedup 1.0000x reference)
"""Trainium2 Bass kernel for DiffusionSelfAttention (B=2, N=2048, A=256, H=8).

Sharding: one attention head per NeuronCore (8 heads / 8 cores).
Per-core program (SPMD, data differs per core):
  - projections q/k/v/gate on PE from host-transposed activations
  - transposed-logits attention: logitsT[k,q] = kT.T @ qT with 4-way
    row-tiled K=32 matmuls (tile_position)
  - softmax via exp(qk)*exp(nbias)*exp(bias): exp(nbias) is DMA'd in fp16
    ("exp-domain" bias), exp(bias) is folded into the PV value matrix and
    the denominator-sum matmul weights, so ACT does a single pure-Exp pass
  - PV numerator + denominator accumulate in one PSUM bank across 4
    concurrent PE column strips
Host: layout transposes, exp of the bias tensors, final normalize+gate.
"""

import os
import sys

for _p in ("/opt/trn_rl_repo",):
    if _p not in sys.path and os.path.isdir(_p):
        sys.path.insert(0, _p)

from contextlib import ExitStack

import numpy as np

import concourse.bass as bass
import concourse.bacc as bacc
import concourse.mybir as mybir
from concourse.bass_utils import run_bass_kernel_spmd
from concourse.tile import TileContext

F16 = mybir.dt.float16
F32 = mybir.dt.float32
AF = mybir.ActivationFunctionType

B, A, H, KD = 2, 256, 8, 32
P = 128
QC = 512          # q columns per psum bank / matmul
N_CORES = 8

# tuning knobs
ROW_TILE_QK = True    # 4-way row-tiled QK matmuls
POOL_MUL_EVERY = 0    # every i-th big elementwise mul goes to gpsimd (0=off)
PL_BUFS = 3
E1_BUFS = 4
E2_BUFS = 4


def build_nc(N=2048):
    NT = N // P            # k tiles of 128
    NG = NT // 4           # k groups of 4 tiles
    NQC = N // QC          # q chunks of 512
    nc = bacc.Bacc("TRN2", target_bir_lowering=False, debug=False)

    qdT = nc.declare_dram_parameter("qdT", [P, B, 2, N], F16, False)
    wcat = nc.declare_dram_parameter("wcat", [P, 2, 320], F16, False)
    qbrep = nc.declare_dram_parameter("qbrep", [P, 1], F32, False)
    cexp = nc.declare_dram_parameter("cexp", [P, B, NT], F32, False)
    e2 = nc.declare_dram_parameter("e2", [NT, P, NQC, QC], F16, False)
    ident = nc.declare_dram_parameter("ident", [P, P], F16, False)
    poraw = nc.declare_dram_parameter("poraw", [B, NQC, 66, QC], F32, True)
    gout = nc.declare_dram_parameter("gout", [B, KD, N], F32, True)

    with TileContext(nc) as tc, ExitStack() as ctx:
        consts = ctx.enter_context(tc.tile_pool(name="consts", bufs=1))
        persist = ctx.enter_context(tc.tile_pool(name="persist", bufs=1))

        wcat_sb = consts.tile([P, 2, 320], F16)
        nc.sync.dma_start(wcat_sb[:], wcat[:])
        ident_sb = consts.tile([P, P], F16)
        nc.sync.dma_start(ident_sb[:], ident[:])
        qbrep_sb = consts.tile([P, 1], F32)
        nc.sync.dma_start(qbrep_sb[:], qbrep[:])
        cexp_sb = consts.tile([P, B, NT], F32)
        nc.sync.dma_start(cexp_sb[:], cexp[:])
        ccol16_sb = consts.tile([P, B, NT], F16)
        nc.vector.tensor_copy(ccol16_sb[:], cexp_sb[:])
        qdT_sb = persist.tile([P, B, 2, N], F16)
        nc.sync.dma_start(qdT_sb[:], qdT[:])

        qT_sb = persist.tile([P, B, N], F16)     # 4x replicated q^T (c on partitions)
        kT_sb = persist.tile([P, B, N], F16)     # 4x replicated k^T
        v_sb = persist.tile([P, B, NT, KD], F16)  # v * exp(bias), natural [k, v]
        gT_sb = persist.tile([64, B, N], F32)     # sigmoid gate, rows 32:64
        vt_tmp = persist.tile([KD, B, N], F16)    # v^T staging for PE transpose

        # ---- prologue: projections ----
        with tc.tile_pool(name="proj_psum", bufs=2, space="PSUM") as projp:
            for b in range(B):
                for nq in range(NQC):
                    sl = slice(nq * QC, (nq + 1) * QC)
                    psq = projp.tile([P, QC], F32, tag="psq")
                    psk = projp.tile([P, QC], F32, tag="psk")
                    psvg = projp.tile([64, QC], F32, tag="psvg")
                    for c in range(2):
                        st, sp = (c == 0), (c == 1)
                        rhs = qdT_sb[:, b, c, sl]
                        nc.tensor.matmul(psq[:], wcat_sb[:, c, 0:128], rhs, start=st, stop=sp)
                        nc.tensor.matmul(psk[:], wcat_sb[:, c, 128:256], rhs, start=st, stop=sp)
                        nc.tensor.matmul(psvg[:], wcat_sb[:, c, 256:320], rhs, start=st, stop=sp)
                    nc.vector.tensor_scalar_add(qT_sb[:, b, sl], psq[:], qbrep_sb[:])
                    nc.vector.tensor_copy(kT_sb[:, b, sl], psk[:])
                    nc.vector.tensor_copy(vt_tmp[:, b, sl], psvg[0:KD])
                    nc.scalar.activation(gT_sb[32:64, b, sl], psvg[32:64], AF.Sigmoid)
                nc.sync.dma_start(gout[b], gT_sb[32:64, b, :])
                # transpose v^T [32, n] -> v [n, 32], fold in exp(bias)
                for t in range(NT):
                    pst = projp.tile([P, KD], F16, tag="pst")
                    nc.tensor.transpose(
                        pst[:], vt_tmp[:, b, t * P:(t + 1) * P], ident_sb[0:KD, 0:KD]
                    )
                    nc.vector.tensor_scalar_mul(
                        v_sb[:, b, t, :], pst[:], cexp_sb[:, b, t:t + 1]
                    )

        # ---- main attention loop ----
        with (
            tc.tile_pool(name="pl_psum", bufs=PL_BUFS, space="PSUM") as plp,
            tc.tile_pool(name="po_psum", bufs=2, space="PSUM") as pop,
            tc.tile_pool(name="sb_main", bufs=E1_BUFS) as sbm,
            tc.tile_pool(name="sb_e2", bufs=E2_BUFS) as sbe2,
            tc.tile_pool(name="sb_out", bufs=2) as sbo,
        ):
            n_mul = 0
            for qc in range(NQC):
                qsl = slice(qc * QC, (qc + 1) * QC)
                po = []
                for b in range(B):
                    pob = pop.tile([P, QC], F32, tag="po")
                    po.append(pob)
                for g in range(NG):
                    e2t = []
                    for u in range(2):
                        t0 = 4 * g + 2 * u
                        et = sbe2.tile([P, 2, QC], F16, tag="e2t")
                        nc.sync.dma_start(
                            et[:], e2[t0:t0 + 2, :, qc, :].rearrange("t p j -> p t j")
                        )
                        e2t.append(et)
                    for b in range(B):
                        pls = []
                        for u in range(2):
                            pl = plp.tile([P, 2, QC], F32, tag="pl")
                            for w in range(2):
                                s = 2 * u + w
                                t = 4 * g + s
                                if ROW_TILE_QK:
                                    nc.tensor.matmul(
                                        pl[:, w, :],
                                        kT_sb[32 * s:32 * s + 32, b, t * P:(t + 1) * P],
                                        qT_sb[32 * s:32 * s + 32, b, qsl],
                                        start=True, stop=True,
                                        tile_position=(32 * s, 0),
                                    )
                                else:
                                    nc.tensor.matmul(
                                        pl[:, w, :],
                                        kT_sb[0:32, b, t * P:(t + 1) * P],
                                        qT_sb[0:32, b, qsl],
                                        start=True, stop=True,
                                    )
                            pls.append(pl)
                        for u in range(2):
                            e1 = sbm.tile([P, 2, QC], F16, tag="e1")
                            nc.scalar.activation(e1[:], pls[u][:], AF.Exp)
                            n_mul += 1
                            eng = (
                                nc.gpsimd
                                if POOL_MUL_EVERY and n_mul % POOL_MUL_EVERY == 0
                                else nc.vector
                            )
                            eng.tensor_mul(e1[:], e1[:], e2t[u][:])
                            for w in range(2):
                                s = 2 * u + w
                                t = 4 * g + s
                                par = s % 2
                                nc.tensor.matmul(
                                    po[b][64 * par:64 * par + 32, :],
                                    v_sb[:, b, t, :],
                                    e1[:, w, :],
                                    start=(g == 0 and s < 2), stop=False,
                                    tile_position=(0, 64 * par),
                                    skip_group_check=True,
                                )
                                nc.tensor.matmul(
                                    po[b][64 * par + 32:64 * par + 33, :],
                                    ccol16_sb[:, b, t:t + 1],
                                    e1[:, w, :],
                                    start=(g == 0 and s < 2),
                                    stop=(g == NG - 1 and s == 3),
                                    tile_position=(0, 64 * par + 32),
                                    skip_group_check=True,
                                )
                for b in range(B):
                    pos = sbo.tile([P, QC], F32, tag="pos")
                    nc.vector.tensor_copy(pos[0:33], po[b][0:33])
                    nc.vector.tensor_copy(pos[64:97], po[b][64:97])
                    nc.sync.dma_start(poraw[b, qc, 0:33], pos[0:33])
                    nc.sync.dma_start(poraw[b, qc, 33:66], pos[64:97])
    nc.compile()
    return nc


def host_prep(q_data, bias, nonbatched_bias, query_w, query_b, key_w, value_w,
              gating_w):
    """Build the per-core input maps (numpy, layout/dtype prep only)."""
    N = q_data.shape[1]
    NT, NQC = N // P, N // QC
    scale = np.float32(KD ** -0.5)
    q_data = np.asarray(q_data, np.float32)
    bias = np.asarray(bias, np.float32)

    # [P, B, 2, N] <- q_data[b, n, 128c+p]
    qdT = np.ascontiguousarray(
        q_data.transpose(2, 0, 1).reshape(2, P, B, N).transpose(1, 2, 0, 3)
    ).astype(np.float16)
    cexp = np.ascontiguousarray(
        np.exp(bias).reshape(B, NT, P).transpose(2, 0, 1)
    ).astype(np.float32)
    identity = np.eye(P, dtype=np.float16)
    qb = np.asarray(query_b, np.float32)[0]          # [H, KD]
    in_maps = []
    for h in range(N_CORES):
        qw = np.asarray(query_w, np.float32)[:, h, :] * scale
        kw = np.asarray(key_w, np.float32)[:, h, :]
        vw = np.asarray(value_w, np.float32)[:, h, :]
        gw = np.asarray(gating_w, np.float32)[:, h, :]
        wall = np.concatenate(
            [np.tile(qw, (1, 4)), np.tile(kw, (1, 4)), vw, gw], axis=1
        )  # [A, 320]
        wcat = np.ascontiguousarray(
            wall.reshape(2, P, 320).transpose(1, 0, 2)
        ).astype(np.float16)
        qbrep = np.tile(qb[h] * scale, 4)[:, None].astype(np.float32)
        e2 = np.exp(np.asarray(nonbatched_bias[h], np.float32).T)  # [k, q]
        e2 = np.ascontiguousarray(e2.reshape(NT, P, NQC, QC)).astype(np.float16)
        in_maps.append({
            "qdT": qdT, "wcat": wcat, "qbrep": qbrep, "cexp": cexp,
            "e2": e2, "ident": identity,
        })
    return in_maps


def host_finish(out_maps, N):
    """Combine per-core raw numerator/denominator into the final output."""
    NQC = N // QC
    out = np.empty((B, N, H, KD), np.float32)
    for h in range(N_CORES):
        po = out_maps[h]["poraw"]           # [B, NQC, 66, QC]
        g = out_maps[h]["gout"]             # [B, KD, N]
        num = po[:, :, 0:32, :] + po[:, :, 33:65, :]     # [B, NQC, 32, QC]
        den = po[:, :, 32, :] + po[:, :, 65, :]          # [B, NQC, QC]
        num = num.transpose(0, 2, 1, 3).reshape(B, KD, N)
        den = den.reshape(B, N)
        o = num / den[:, None, :] * g                     # [B, KD, N]
        out[:, :, h, :] = o.transpose(0, 2, 1)
    return out


_RUN_KWARGS = {}


def kernel(q_data, bias, nonbatched_bias, query_w, query_b, key_w, value_w,
           gating_w):
    N = q_data.shape[1]
    nc = build_nc(N)
    in_maps = host_prep(q_data, bias, nonbatched_bias, query_w, query_b,
                        key_w, value_w, gating_w)
    res = run_bass_kernel_spmd(nc, in_maps, list(range(N_CORES)), **_RUN_KWARGS)
    out = host_finish(res.results, N)
    kernel.last_results = res
    return out


if __name__ == "__main__":
    np.random.seed(0)
    N = 512
    inputs = {
        "q_data": np.random.randn(B, N, A).astype(np.float32),
        "bias": np.random.randn(B, N).astype(np.float32),
        "nonbatched_bias": np.random.randn(H, N, N).astype(np.float32),
        "query_w": (np.random.randn(A, H, KD) * 0.05).astype(np.float32),
        "query_b": (np.random.randn(1, H, KD) * 0.05).astype(np.float32),
        "key_w": (np.random.randn(A, H, KD) * 0.05).astype(np.float32),
        "value_w": (np.random.randn(A, H, KD) * 0.05).astype(np.float32),
        "gating_w": (np.random.randn(A, H, KD) * 0.05).astype(np.float32),
    }
    out = kernel(**inputs)
    print("out", out.shape, out.dtype, np.abs(out).max())


# revision 11
# speedup vs baseline: 75.7009x; 75.7009x over previous
"""Trainium2 Bass kernel for DiffusionSelfAttention (B=2, N=2048, A=256, H=8).

Sharding: one attention head per NeuronCore (8 heads / 8 cores).
Per-core program (SPMD, data differs per core):
  - projections q/k/v/gate on PE from host-transposed activations
  - transposed-logits attention: logitsT[k,q] = kT.T @ qT with 4-way
    row-tiled K=32 matmuls (tile_position)
  - softmax via exp(qk)*exp(nbias)*exp(bias): exp(nbias) is DMA'd in fp16
    ("exp-domain" bias), exp(bias) is folded into the PV value matrix and
    the denominator-sum matmul weights, so ACT does a single pure-Exp pass
  - PV numerator + denominator accumulate in one PSUM bank across 4
    concurrent PE column strips
Host: layout transposes, exp of the bias tensors, final normalize+gate.
"""

import os
import sys

for _p in ("/opt/trn_rl_repo",):
    if _p not in sys.path and os.path.isdir(_p):
        sys.path.insert(0, _p)

from contextlib import ExitStack

import numpy as np

import concourse.bass as bass
import concourse.bacc as bacc
import concourse.mybir as mybir
from concourse.bass_utils import run_bass_kernel_spmd
from concourse.tile import TileContext

F16 = mybir.dt.float16
F32 = mybir.dt.float32
AF = mybir.ActivationFunctionType

B, A, H, KD = 2, 256, 8, 32
P = 128
QC = 512          # q columns per psum bank / matmul
N_CORES = 8

# tuning knobs
ROW_TILE_QK = True    # 4-way row-tiled QK matmuls
POOL_MUL_EVERY = 0    # every i-th big elementwise mul goes to gpsimd (0=off)
PL_BUFS = 3
E1_BUFS = 4
E2_BUFS = 4


def build_nc(N=2048, repeat=1):
    NT = N // P            # k tiles of 128
    NG = NT // 4           # k groups of 4 tiles
    NQC = N // QC          # q chunks of 512
    nc = bacc.Bacc("TRN2", target_bir_lowering=False, debug=False)

    qdT = nc.declare_dram_parameter("qdT", [P, B, 2, N], F16, False)
    wcat = nc.declare_dram_parameter("wcat", [P, 2, 320], F16, False)
    qbrep = nc.declare_dram_parameter("qbrep", [P, 1], F32, False)
    cexp = nc.declare_dram_parameter("cexp", [P, B, NT], F32, False)
    e2 = nc.declare_dram_parameter("e2", [NT, P, NQC, QC], F16, False)
    ident = nc.declare_dram_parameter("ident", [P, P], F16, False)
    poraw = nc.declare_dram_parameter("poraw", [B, NQC, 66, QC], F32, True)
    gout = nc.declare_dram_parameter("gout", [B, KD, N], F32, True)

    with TileContext(nc) as tc:
      for rep in range(repeat):
       with ExitStack() as ctx:
        consts = ctx.enter_context(tc.tile_pool(name=f"consts{rep}", bufs=1))
        persist = ctx.enter_context(tc.tile_pool(name=f"persist{rep}", bufs=1))

        wcat_sb = consts.tile([P, 2, 320], F16)
        nc.sync.dma_start(wcat_sb[:], wcat[:])
        ident_sb = consts.tile([P, P], F16)
        nc.sync.dma_start(ident_sb[:], ident[:])
        qbrep_sb = consts.tile([P, 1], F32)
        nc.sync.dma_start(qbrep_sb[:], qbrep[:])
        cexp_sb = consts.tile([P, B, NT], F32)
        nc.sync.dma_start(cexp_sb[:], cexp[:])
        ccol16_sb = consts.tile([P, B, NT], F16)
        nc.vector.tensor_copy(ccol16_sb[:], cexp_sb[:])
        qdT_sb = persist.tile([P, B, 2, N], F16)
        nc.sync.dma_start(qdT_sb[:], qdT[:])

        qT_sb = persist.tile([P, B, N], F16)     # 4x replicated q^T (c on partitions)
        kT_sb = persist.tile([P, B, N], F16)     # 4x replicated k^T
        v_sb = persist.tile([P, B, NT, KD], F16)  # v * exp(bias), natural [k, v]
        gT_sb = persist.tile([64, B, N], F32)     # sigmoid gate, rows 32:64
        vt_tmp = persist.tile([KD, B, N], F16)    # v^T staging for PE transpose

        # ---- prologue: projections ----
        with tc.tile_pool(name=f"proj_psum{rep}", bufs=2, space="PSUM") as projp:
            for b in range(B):
                for nq in range(NQC):
                    sl = slice(nq * QC, (nq + 1) * QC)
                    psq = projp.tile([P, QC], F32, tag="psq")
                    psk = projp.tile([P, QC], F32, tag="psk")
                    psvg = projp.tile([64, QC], F32, tag="psvg")
                    for c in range(2):
                        st, sp = (c == 0), (c == 1)
                        rhs = qdT_sb[:, b, c, sl]
                        nc.tensor.matmul(psq[:], wcat_sb[:, c, 0:128], rhs, start=st, stop=sp)
                        nc.tensor.matmul(psk[:], wcat_sb[:, c, 128:256], rhs, start=st, stop=sp)
                        nc.tensor.matmul(psvg[:], wcat_sb[:, c, 256:320], rhs, start=st, stop=sp)
                    nc.vector.tensor_scalar_add(qT_sb[:, b, sl], psq[:], qbrep_sb[:])
                    nc.vector.tensor_copy(kT_sb[:, b, sl], psk[:])
                    nc.vector.tensor_copy(vt_tmp[:, b, sl], psvg[0:KD])
                    nc.scalar.activation(gT_sb[32:64, b, sl], psvg[32:64], AF.Sigmoid)
                nc.sync.dma_start(gout[b], gT_sb[32:64, b, :])
                # transpose v^T [32, n] -> v [n, 32], fold in exp(bias)
                for t in range(NT):
                    pst = projp.tile([P, KD], F16, tag="pst")
                    nc.tensor.transpose(
                        pst[:], vt_tmp[:, b, t * P:(t + 1) * P], ident_sb[0:KD, 0:KD]
                    )
                    nc.vector.tensor_scalar_mul(
                        v_sb[:, b, t, :], pst[:], cexp_sb[:, b, t:t + 1]
                    )

        # ---- main attention loop ----
        with (
            tc.tile_pool(name=f"pl_psum{rep}", bufs=PL_BUFS, space="PSUM") as plp,
            tc.tile_pool(name=f"po_psum{rep}", bufs=2, space="PSUM") as pop,
            tc.tile_pool(name=f"sb_main{rep}", bufs=E1_BUFS) as sbm,
            tc.tile_pool(name=f"sb_e2{rep}", bufs=E2_BUFS) as sbe2,
            tc.tile_pool(name=f"sb_out{rep}", bufs=2) as sbo,
        ):
            n_mul = 0
            for qc in range(NQC):
                qsl = slice(qc * QC, (qc + 1) * QC)
                po = []
                for b in range(B):
                    pob = pop.tile([P, QC], F32, tag="po")
                    po.append(pob)
                for g in range(NG):
                    e2t = []
                    for u in range(2):
                        t0 = 4 * g + 2 * u
                        et = sbe2.tile([P, 2, QC], F16, tag="e2t")
                        nc.sync.dma_start(
                            et[:], e2[t0:t0 + 2, :, qc, :].rearrange("t p j -> p t j")
                        )
                        e2t.append(et)
                    for b in range(B):
                        pls = []
                        for u in range(2):
                            pl = plp.tile([P, 2, QC], F32, tag="pl")
                            for w in range(2):
                                s = 2 * u + w
                                t = 4 * g + s
                                if ROW_TILE_QK:
                                    nc.tensor.matmul(
                                        pl[:, w, :],
                                        kT_sb[32 * s:32 * s + 32, b, t * P:(t + 1) * P],
                                        qT_sb[32 * s:32 * s + 32, b, qsl],
                                        start=True, stop=True,
                                        tile_position=(32 * s, 0),
                                    )
                                else:
                                    nc.tensor.matmul(
                                        pl[:, w, :],
                                        kT_sb[0:32, b, t * P:(t + 1) * P],
                                        qT_sb[0:32, b, qsl],
                                        start=True, stop=True,
                                    )
                            pls.append(pl)
                        for u in range(2):
                            e1 = sbm.tile([P, 2, QC], F16, tag="e1")
                            nc.scalar.activation(e1[:], pls[u][:], AF.Exp)
                            n_mul += 1
                            eng = (
                                nc.gpsimd
                                if POOL_MUL_EVERY and n_mul % POOL_MUL_EVERY == 0
                                else nc.vector
                            )
                            eng.tensor_mul(e1[:], e1[:], e2t[u][:])
                            for w in range(2):
                                s = 2 * u + w
                                t = 4 * g + s
                                par = s % 2
                                nc.tensor.matmul(
                                    po[b][64 * par:64 * par + 32, :],
                                    v_sb[:, b, t, :],
                                    e1[:, w, :],
                                    start=(g == 0 and s < 2), stop=False,
                                    tile_position=(0, 64 * par),
                                    skip_group_check=True,
                                )
                                nc.tensor.matmul(
                                    po[b][64 * par + 32:64 * par + 33, :],
                                    ccol16_sb[:, b, t:t + 1],
                                    e1[:, w, :],
                                    start=(g == 0 and s < 2),
                                    stop=(g == NG - 1 and s == 3),
                                    tile_position=(0, 64 * par + 32),
                                    skip_group_check=True,
                                )
                for b in range(B):
                    pos = sbo.tile([P, QC], F32, tag="pos")
                    nc.vector.tensor_copy(pos[0:33], po[b][0:33])
                    nc.vector.tensor_copy(pos[64:97], po[b][64:97])
                    nc.sync.dma_start(poraw[b, qc, 0:33], pos[0:33])
                    nc.sync.dma_start(poraw[b, qc, 33:66], pos[64:97])
    nc.compile()
    return nc


def host_prep(q_data, bias, nonbatched_bias, query_w, query_b, key_w, value_w,
              gating_w):
    """Build the per-core input maps (numpy, layout/dtype prep only)."""
    N = q_data.shape[1]
    NT, NQC = N // P, N // QC
    scale = np.float32(KD ** -0.5)
    q_data = np.asarray(q_data, np.float32)
    bias = np.asarray(bias, np.float32)

    # [P, B, 2, N] <- q_data[b, n, 128c+p]
    qdT = np.ascontiguousarray(
        q_data.transpose(2, 0, 1).reshape(2, P, B, N).transpose(1, 2, 0, 3)
    ).astype(np.float16)
    cexp = np.ascontiguousarray(
        np.exp(bias).reshape(B, NT, P).transpose(2, 0, 1)
    ).astype(np.float32)
    identity = np.eye(P, dtype=np.float16)
    qb = np.asarray(query_b, np.float32)[0]          # [H, KD]
    in_maps = []
    for h in range(N_CORES):
        qw = np.asarray(query_w, np.float32)[:, h, :] * scale
        kw = np.asarray(key_w, np.float32)[:, h, :]
        vw = np.asarray(value_w, np.float32)[:, h, :]
        gw = np.asarray(gating_w, np.float32)[:, h, :]
        wall = np.concatenate(
            [np.tile(qw, (1, 4)), np.tile(kw, (1, 4)), vw, gw], axis=1
        )  # [A, 320]
        wcat = np.ascontiguousarray(
            wall.reshape(2, P, 320).transpose(1, 0, 2)
        ).astype(np.float16)
        qbrep = np.tile(qb[h] * scale, 4)[:, None].astype(np.float32)
        e2 = np.exp(np.asarray(nonbatched_bias[h], np.float32).T)  # [k, q]
        e2 = np.ascontiguousarray(e2.reshape(NT, P, NQC, QC)).astype(np.float16)
        in_maps.append({
            "qdT": qdT, "wcat": wcat, "qbrep": qbrep, "cexp": cexp,
            "e2": e2, "ident": identity,
        })
    return in_maps


def host_finish(out_maps, N):
    """Combine per-core raw numerator/denominator into the final output."""
    NQC = N // QC
    out = np.empty((B, N, H, KD), np.float32)
    for h in range(N_CORES):
        po = out_maps[h]["poraw"]           # [B, NQC, 66, QC]
        g = out_maps[h]["gout"]             # [B, KD, N]
        num = po[:, :, 0:32, :] + po[:, :, 33:65, :]     # [B, NQC, 32, QC]
        den = po[:, :, 32, :] + po[:, :, 65, :]          # [B, NQC, QC]
        num = num.transpose(0, 2, 1, 3).reshape(B, KD, N)
        den = den.reshape(B, N)
        o = num / den[:, None, :] * g                     # [B, KD, N]
        out[:, :, h, :] = o.transpose(0, 2, 1)
    return out


_RUN_KWARGS = {}


def kernel(q_data, bias, nonbatched_bias, query_w, query_b, key_w, value_w,
           gating_w):
    N = q_data.shape[1]
    nc = build_nc(N)
    in_maps = host_prep(q_data, bias, nonbatched_bias, query_w, query_b,
                        key_w, value_w, gating_w)
    res = run_bass_kernel_spmd(nc, in_maps, list(range(N_CORES)), **_RUN_KWARGS)
    out = host_finish(res.results, N)
    kernel.last_results = res
    return out


if __name__ == "__main__":
    np.random.seed(0)
    N = 512
    inputs = {
        "q_data": np.random.randn(B, N, A).astype(np.float32),
        "bias": np.random.randn(B, N).astype(np.float32),
        "nonbatched_bias": np.random.randn(H, N, N).astype(np.float32),
        "query_w": (np.random.randn(A, H, KD) * 0.05).astype(np.float32),
        "query_b": (np.random.randn(1, H, KD) * 0.05).astype(np.float32),
        "key_w": (np.random.randn(A, H, KD) * 0.05).astype(np.float32),
        "value_w": (np.random.randn(A, H, KD) * 0.05).astype(np.float32),
        "gating_w": (np.random.randn(A, H, KD) * 0.05).astype(np.float32),
    }
    out = kernel(**inputs)
    print("out", out.shape, out.dtype, np.abs(out).max())
